# revision 13
# baseline (speedup 1.0000x reference)
"""Trainium2 Bass kernel for a 3-layer LSTM (B=64, T=256, F=64, H=1024)
+ tanh output projection, SPMD across 8 NeuronCores.

Strategy: shard the 4H=4096 gate dimension 8 ways (each core owns a
512-wide gate slice == a 128-wide h-slice per layer), full batch B=64
on every core. The 3 layers are software-pipelined: slot s computes
L1(t=s), L2(t=s-1), L3(t=s-2).

vs the original version:
- all matmuls are bf16 (fp32 matmul runs as 2 half-rate passes on TRN2,
  so this is ~4x PE throughput); PSUM accumulation stays fp32
- gate biases are folded into the matmuls (L1 via a constant-1 row in
  the zero-padded X k-tile, L2/L3 via one K=2 ones-matmul), so the
  activations read gate PSUM directly and the DVE bias-add disappears
- ONE packed AllGather per slot ([128, 192] bf16 carrying h1(s),
  h2(s-1), h3(s-2)) instead of two fp32 collectives - half the wire
  bytes, half the collective launches
- the gathered buffer [128, 8, 192] is consumed directly as matmul
  lhsT k-tiles (no per-layer re-fetch)
- L1's gate PSUM sits in partitions 64:127 so its matmuls run in PE
  column group B concurrently with L2's in group A
"""

import sys

sys.path.insert(0, "/opt/trn_rl_repo")

import numpy as np
import ml_dtypes

BF16 = ml_dtypes.bfloat16

B, T, F, H = 64, 256, 64, 1024
R = 8           # cores
G = 4 * H // R  # 512 gate slice per core
HS = H // R     # 128 h slice per core
YG = 8          # y-projection group size (slots)
NB = 6          # gather ring depth
W23 = 3 * B     # packed exchange width (h1|h2|h3)

_GATE_ORDER = (0, 1, 3, 2)  # i, f, o, g (PyTorch row blocks i,f,g,o)


def _slice_rows(w, r):
    """Rows of a [4H, *] weight for core r, in i|f|o|g block order."""
    return np.concatenate([w[g * H + HS * r: g * H + HS * (r + 1)] for g in _GATE_ORDER], 0)


def _k_tiles(wT):
    """[K, G] -> [128, K/128, G] SBUF layout (partition-major k-tiles)."""
    K = wT.shape[0]
    return np.ascontiguousarray(
        wT.reshape(K // 128, 128, wT.shape[1]).transpose(1, 0, 2)
    )


def _prep_core_inputs(r, X, weights):
    (w_ih1, w_hh1, b_ih1, b_hh1, w_ih2, w_hh2, b_ih2, b_hh2,
     w_ih3, w_hh3, b_ih3, b_hh3, w_out, b_out) = weights
    inp = {}
    # X: [B, T, F] -> [T, 128(pad F), B] with constant-1 row at F (bias row)
    Xt = np.zeros((T, 128, B), np.float32)
    Xt[:, :F, :] = X.transpose(1, 2, 0)
    Xt[:, F, :] = 1.0
    inp["Xt"] = Xt.astype(BF16)
    # layer 1 input weight: [512, F].T -> [F, 512]; row F = bias slice
    w1 = _slice_rows(w_ih1, r).T  # [F, 512]
    W1x = np.zeros((128, G), np.float32)
    W1x[:F] = w1
    W1x[F] = _slice_rows((b_ih1 + b_hh1).reshape(4 * H, 1), r)[:, 0]
    inp["W1x"] = W1x.astype(BF16)
    for name, w in (("Whh1", w_hh1), ("Wih2", w_ih2), ("Whh2", w_hh2),
                    ("Wih3", w_ih3), ("Whh3", w_hh3)):
        inp[name] = _k_tiles(_slice_rows(w, r).T.astype(np.float32)).astype(BF16)

    def brow(bi, bh):
        return _slice_rows((bi + bh).reshape(4 * H, 1), r)[:, 0]  # [512]

    inp["bias23"] = np.stack([brow(b_ih2, b_hh2), brow(b_ih3, b_hh3)]).astype(BF16)
    ones2 = np.zeros((2, 128), np.float32)
    ones2[0, 0:64] = 1.0
    ones2[1, 64:128] = 1.0
    inp["ones2"] = ones2.astype(BF16)
    # output projection: w_out [F, H] -> lhsT tiles [128, 8, F]
    inp["Wout"] = _k_tiles(np.ascontiguousarray(w_out.T).astype(np.float32)).astype(BF16)
    inp["bout"] = b_out.reshape(F, 1).astype(np.float32)
    return inp


def build_nc(t_steps=T, reps=1, y_small=False):
    import concourse.bass as bass
    import concourse.mybir as mybir
    import concourse.tile as tile
    from concourse import bacc
    from concourse.masks import make_identity

    f32 = mybir.dt.float32
    bf16 = mybir.dt.bfloat16
    AF = mybir.ActivationFunctionType
    NSLOT = t_steps + 3
    NEX = t_steps + 2   # exchanges per rep (slots 0..T+1)
    rg = [list(range(R))]

    nc = bacc.Bacc("TRN2", target_bir_lowering=False, debug=False, num_devices=R)

    p_Xt = nc.dram_tensor("Xt", [T, 128, B], bf16, kind="ExternalInput")
    p_W1x = nc.dram_tensor("W1x", [128, G], bf16, kind="ExternalInput")
    pw = {}
    for name in ("Whh1", "Wih2", "Whh2", "Wih3", "Whh3"):
        pw[name] = nc.dram_tensor(name, [128, 8, G], bf16, kind="ExternalInput")
    p_b23 = nc.dram_tensor("bias23", [2, G], bf16, kind="ExternalInput")
    p_ones2 = nc.dram_tensor("ones2", [2, 128], bf16, kind="ExternalInput")
    p_Wout = nc.dram_tensor("Wout", [128, 8, F], bf16, kind="ExternalInput")
    p_bout = nc.dram_tensor("bout", [F, 1], f32, kind="ExternalInput")
    ycols = YG * B if y_small else t_steps * B
    p_Y = nc.dram_tensor("Y", [F, ycols], f32, kind="ExternalOutput")

    with tile.TileContext(nc) as tc:
        with (
            tc.tile_pool(name="wpool", bufs=1) as wpool,
            tc.tile_pool(name="state", bufs=1) as state,
            tc.tile_pool(name="xq", bufs=4) as xq,
            tc.tile_pool(name="sbt", bufs=3) as sbt,
            tc.tile_pool(name="h3g", bufs=2) as h3g,
            tc.tile_pool(name="gps", bufs=4, space="PSUM") as gps,
            tc.tile_pool(name="tps", bufs=2, space="PSUM") as tps,
            tc.tile_pool(name="yps", bufs=1, space="PSUM") as yps,
            tc.tile_pool(name="dms", bufs=4, space="DRAM") as dms,
        ):
            # ---- resident weights ----
            W1x = wpool.tile([128, G], bf16, tag="W1x")
            nc.sync.dma_start(W1x[:], p_W1x[:])
            W = {}
            for name in ("Whh1", "Wih2", "Whh2", "Wih3", "Whh3"):
                W[name] = wpool.tile([128, 8, G], bf16, tag=name, name=name + "_sb")
                nc.sync.dma_start(W[name][:], pw[name][:])
            b23 = wpool.tile([2, G], bf16, tag="b23", name="b23_sb")
            nc.sync.dma_start(b23[:], p_b23[:])
            ones2 = wpool.tile([2, 128], bf16, tag="ones2", name="ones2_sb")
            nc.sync.dma_start(ones2[:], p_ones2[:])
            Wout = wpool.tile([128, 8, F], bf16, tag="Wout")
            nc.sync.dma_start(Wout[:], p_Wout[:])
            bout = wpool.tile([F, 1], f32, tag="bout")
            nc.sync.dma_start(bout[:], p_bout[:])
            ident = wpool.tile([128, 128], bf16, tag="ident")
            make_identity(nc, ident[:])

            # ---- exchange buffers ----
            stg = [wpool.tile([128, W23], bf16, tag=f"stg{i}", name=f"stg{i}")
                   for i in range(2)]
            for t_ in stg:
                nc.vector.memset(t_[:], 0.0)
            Hgp = [wpool.tile([128, R, W23], bf16, tag=f"Hgp{i}", name=f"Hgp{i}")
                   for i in range(NB)]

            # ---- persistent state ----
            c1t = state.tile([128, HS], f32, tag="c1", name="c1")   # [64:128] used
            c23t = state.tile([128, HS], f32, tag="c23", name="c23")

            def lstm_ew(key, gpsum, c, P, hbase):
                """gates psum [P, G] (i|f|o|g) + c [P, HS] -> h bf16 [P, HS]."""
                sio = sbt.tile([128, 3 * HS], f32, tag=f"sio{key}",
                               name=f"sio{key}")[hbase:hbase + P]
                nc.scalar.activation(sio, gpsum[:, 0:3 * HS], AF.Sigmoid)
                tg = sbt.tile([128, HS], f32, tag=f"tg{key}",
                              name=f"tg{key}")[hbase:hbase + P]
                nc.scalar.activation(tg, gpsum[:, 3 * HS:4 * HS], AF.Tanh)
                fc = sbt.tile([128, HS], f32, tag=f"fc{key}",
                              name=f"fc{key}")[hbase:hbase + P]
                nc.vector.tensor_mul(out=fc, in0=sio[:, HS:2 * HS], in1=c)
                ig = sbt.tile([128, HS], f32, tag=f"ig{key}",
                              name=f"ig{key}")[hbase:hbase + P]
                nc.vector.tensor_mul(out=ig, in0=sio[:, 0:HS], in1=tg)
                nc.vector.tensor_add(out=c, in0=fc, in1=ig)
                tc_ = sbt.tile([128, HS], f32, tag=f"tc{key}",
                               name=f"tc{key}")[hbase:hbase + P]
                nc.scalar.activation(tc_, c, AF.Tanh)
                h = sbt.tile([128, HS], bf16, tag=f"h{key}",
                             name=f"h{key}")[hbase:hbase + P]
                nc.vector.tensor_mul(out=h, in0=sio[:, 2 * HS:3 * HS], in1=tc_)
                return h

            cur_grp = [None]

            for rep_s in range(reps * NSLOT):
                s = rep_s % NSLOT
                rep = rep_s // NSLOT
                if s == 0:
                    nc.vector.memset(c1t[64:128], 0.0)
                    nc.vector.memset(c23t[:], 0.0)
                ex_base = rep * NEX
                cons = Hgp[(ex_base + s - 1) % NB]  # exchange issued at slot s-1
                l1_active = s < t_steps
                t2, t3 = s - 1, s - 2
                l2_active = 0 <= t2 < t_steps
                l3_active = 0 <= t3 < t_steps

                # ---------- layer 1 x-side matmul (no exchange dependency) --
                if l1_active:
                    xs = xq.tile([128, B], bf16)
                    nc.sync.dma_start(xs[:], p_Xt[s])
                    g1t = gps.tile([128, G], f32, tag="g", name="g1")
                    g1 = g1t[64:128]
                    nc.tensor.matmul(g1, xs[:], W1x[:], start=True, stop=(s == 0))

                # ---------- layers 2+3 matmuls into g23 [128, G] ----------
                if l2_active or l3_active:
                    g23t = gps.tile([128, G], f32, tag="g", name="g23")
                    nc.tensor.matmul(g23t[:], ones2[:], b23[:], start=True,
                                     stop=False, skip_group_check=True)
                    gl2 = g23t[0:B]
                    gl3 = g23t[B:2 * B]
                    l2_mms = []
                    l3_mms = []
                    if l2_active:
                        l2_mms += [(cons[:, k, 0:B], W["Wih2"][:, k]) for k in range(8)]
                        if t2 > 0:
                            l2_mms += [(cons[:, k, B:2 * B], W["Whh2"][:, k])
                                       for k in range(8)]
                    if l3_active:
                        l3_mms += [(cons[:, k, B:2 * B], W["Wih3"][:, k])
                                   for k in range(8)]
                        if t3 > 0:
                            l3_mms += [(cons[:, k, 2 * B:3 * B], W["Whh3"][:, k])
                                       for k in range(8)]
                    n = max(len(l2_mms), len(l3_mms))
                    for i in range(n):
                        if i < len(l2_mms):
                            lhsT, rhs = l2_mms[i]
                            nc.tensor.matmul(gl2, lhsT, rhs, start=False,
                                             stop=(i == len(l2_mms) - 1),
                                             skip_group_check=True)
                        if i < len(l3_mms):
                            lhsT, rhs = l3_mms[i]
                            nc.tensor.matmul(gl3, lhsT, rhs, start=False,
                                             stop=(i == len(l3_mms) - 1),
                                             skip_group_check=True)

                # L1 recurrent matmuls AFTER L2/L3's: the g23->ew23->exchange
                # chain is the critical path; L1's run in its ACT/AG shadow.
                if l1_active and s > 0:
                    for k in range(8):
                        nc.tensor.matmul(g1, cons[:, k, 0:B], W["Whh1"][:, k],
                                         start=False, stop=(k == 7))

                # ---------- elementwise + transpose + staged exchange -------
                do_ex = s <= t_steps + 1
                if do_ex:
                    sb = stg[(ex_base + s) % 2]
                if l1_active:
                    h1 = lstm_ew("1", g1, c1t[64:128], B, 64)
                    pt1 = tps.tile([128, 1024], bf16, tag="pt", name="pt1")[:, 0:B]
                    nc.tensor.transpose(pt1[:], h1, ident[64:128, 64:128])
                    nc.vector.tensor_copy(out=sb[:, 0:B], in_=pt1[:])
                if l2_active or l3_active:
                    if l2_active and l3_active:
                        h23 = lstm_ew("23", g23t[:], c23t[:], 2 * B, 0)
                        tp_in, tp_out0, tp_p, ib = h23, 0, 2 * B, 0
                    elif l2_active:   # s == 1
                        h23 = lstm_ew("2", gl2, c23t[0:B], B, 0)
                        tp_in, tp_out0, tp_p, ib = h23, 0, B, 0
                    else:             # s == t_steps + 1, L3 solo
                        h23 = lstm_ew("3", gl3, c23t[B:2 * B], B, 64)
                        tp_in, tp_out0, tp_p, ib = h23, B, B, 64
                    pt23 = tps.tile([128, 1024], bf16, tag="pt",
                                    name="pt23")[:, 0:2 * B]
                    nc.tensor.transpose(
                        pt23[:, tp_out0:tp_out0 + tp_p], tp_in,
                        ident[ib:ib + tp_p, ib:ib + tp_p])
                    nc.vector.tensor_copy(
                        out=sb[:, B + tp_out0:B + tp_out0 + tp_p],
                        in_=pt23[:, tp_out0:tp_out0 + tp_p])

                if do_ex:
                    agin = dms.tile([128, W23], bf16, tag="agin", name="agin")
                    nc.sync.dma_start(agin[:, B:3 * B], sb[:, B:3 * B])
                    nc.scalar.dma_start(agin[:, 0:B], sb[:, 0:B])
                    agout = dms.tile([R, 128, W23], bf16, tag="agout", name="agout")
                    nc.gpsimd.collective_compute(
                        "AllGather", mybir.AluOpType.bypass,
                        replica_groups=rg, ins=[agin[:].opt()], outs=[agout[:].opt()],
                    )
                    nxt = Hgp[(ex_base + s) % NB]
                    # h1 part lands first (it heads the next slot's recurrence);
                    # h2/h3 parts follow on a different queue
                    nc.gpsimd.dma_start(
                        nxt[:, 0:4, 0:B],
                        agout[0:4, :, 0:B].rearrange("r p w -> p r w"))
                    nc.gpsimd.dma_start(
                        nxt[:, 4:8, 0:B],
                        agout[4:8, :, 0:B].rearrange("r p w -> p r w"))
                    nc.scalar.dma_start(
                        nxt[:, :, B:3 * B],
                        agout[:, :, B:3 * B].rearrange("r p w -> p r w"))

                # ---------- h3 group copy (t = s-3) + projection ----------
                tg3 = s - 3
                if 0 <= tg3 < t_steps:
                    gidx, j = tg3 // YG, tg3 % YG
                    if j == 0:
                        cur_grp[0] = h3g.tile([128, 8, YG, B], bf16,
                                              tag="h3grp", name="h3grp")
                    grp = cur_grp[0]
                    nc.sync.dma_start(grp[:, :, j, :], cons[:, :, 2 * B:3 * B])
                    if j == YG - 1:
                        yp = yps.tile([F, YG * B], f32)
                        for k in range(8):
                            nc.tensor.matmul(yp[:], Wout[:, k], grp[:, k],
                                             start=(k == 0), stop=(k == 7))
                        ysb = sbt.tile([F, YG * B], f32, tag="ysb")
                        nc.scalar.activation(ysb[:], yp[:], AF.Tanh, bias=bout[:])
                        y0 = 0 if y_small else gidx * YG * B
                        nc.sync.dma_start(p_Y[:, y0:y0 + YG * B], ysb[:])

    nc.compile()
    return nc


_CACHED = {}


def _get_nc(t_steps=T):
    if t_steps not in _CACHED:
        _CACHED[t_steps] = build_nc(t_steps)
    return _CACHED[t_steps]


def make_in_maps(X, weights):
    return [_prep_core_inputs(r, X, weights) for r in range(R)]


def _weights_tuple(kw):
    return tuple(
        np.asarray(kw[k], np.float32)
        for k in ("w_ih1", "w_hh1", "b_ih1", "b_hh1", "w_ih2", "w_hh2", "b_ih2",
                  "b_hh2", "w_ih3", "w_hh3", "b_ih3", "b_hh3", "w_out", "b_out")
    )


def assemble_output(Y, t_steps=T):
    """[F, t*B] -> [B, t, F]"""
    return np.ascontiguousarray(Y.reshape(F, t_steps, B).transpose(2, 1, 0))


def kernel(X, **kw):
    from concourse.bass_utils import run_bass_kernel_spmd

    nc = _get_nc(T)
    in_maps = make_in_maps(np.asarray(X, np.float32), _weights_tuple(kw))
    res = run_bass_kernel_spmd(nc, in_maps, core_ids=list(range(R)))
    return assemble_output(res.results[0]["Y"])
